# revision 25
# baseline (speedup 1.0000x reference)
"""Transposed-S ("column softmax") Bass kernel — see kernel docstring in
kernel.py for the shared math. Differences vs the row formulation:
no A-transposes (64/image) and no selector matmul (16 passes/image); instead
a t^T identity-matmul (8 x 128-free passes), DVE/GpSimd broadcast bias adds
along the free dim, and a ones-matmul softmax denominator (16 passes).
Net TensorEngine work: -6.7K cycles/image (~-6%).

Startup DMA trigger sequence is byte-identical to the validated-fast baseline
(GS still loaded though unused) — changing it drops the PE clock bin 1.2x.
"""

import sys

if "/opt/trn_rl_repo" not in sys.path:
    sys.path.insert(0, "/opt/trn_rl_repo")

import numpy as np

import concourse.bass as bass
import concourse.tile as tile
from concourse import bacc, mybir
from concourse.bass_utils import run_bass_kernel_spmd
from concourse.masks import make_identity

F32 = mybir.dt.float32
F32R = mybir.dt.float32r
BF16 = mybir.dt.bfloat16

B, C, H, W_ = 64, 512, 32, 32
N = H * W_              # 1024 positions
NCORES = 8
BLOC = B // NCORES      # 8 images per core
CT = C // 128           # 4 channel tiles
NT = N // 128           # 8 position tiles
P = 128

IDENT_F32_BITCAST = True  # build identity as F32 (known-good affine_select)


def _round_f32r(a):
    from neuronxcc.starfish.support.dtype import static_cast_fp32_to_fp32r
    return np.asarray(static_cast_fp32_to_fp32r(
        np.ascontiguousarray(a, dtype=np.float32))).view(np.float32)


def build_nc():
    bloc = BLOC
    nc = bacc.Bacc("TRN2", target_bir_lowering=False, debug=False,
                   num_devices=NCORES)
    x_ext = nc.declare_dram_parameter("x", [bloc, C, N], F32R, isOutput=False)
    mt_ext = nc.declare_dram_parameter("MT", [C, C], F32R, isOutput=False)
    wvt_ext = nc.declare_dram_parameter("WVT", [C, C], F32R, isOutput=False)
    pf_ext = nc.declare_dram_parameter("PF", [C, P], F32R, isOutput=False)
    gs_ext = nc.declare_dram_parameter("GS", [P, N], F32R, isOutput=False)
    o_ext = nc.declare_dram_parameter("out", [bloc, C, N], F32, isOutput=True)

    with tile.TileContext(nc) as tc:
        with (
            tc.tile_pool(name="const", bufs=1) as const,
            tc.tile_pool(name="wt", bufs=1) as wtp,
            tc.tile_pool(name="xf", bufs=3) as xfp,
            tc.tile_pool(name="qk", bufs=1) as qkp,
            tc.tile_pool(name="vt", bufs=1) as vtp,
            tc.tile_pool(name="sf", bufs=3) as sfp,
            tc.tile_pool(name="et", bufs=1) as etp,
            tc.tile_pool(name="osb", bufs=4) as osbp,
            tc.tile_pool(name="rp", bufs=2) as rpp,
            tc.tile_pool(name="pbig", bufs=2, space="PSUM") as pbig,
            tc.tile_pool(name="pv", bufs=2, space="PSUM") as pvp,
            tc.tile_pool(name="psr", bufs=1, space="PSUM") as psrp,
        ):
            # walrus requires f32r matmul operands to be f32r-rounded by
            # their producer; affine_select (make_identity) is not, so pass
            # the identity through a DVE cast-copy into an F32R tile.
            ident_host = const.tile([P, P], F32, tag="idf")
            make_identity(nc, ident_host[:])
            ident_rt = const.tile([P, P], F32R, tag="idr")
            nc.vector.tensor_copy(ident_rt[:], ident_host[:])
            ident_f32r = ident_rt[:]
            nbias = const.tile([P, 1], F32, tag="nbias")
            nc.vector.memset(nbias[:], -90.0)
            ones_bf = const.tile([P, P], BF16, tag="ones")
            nc.vector.memset(ones_bf[:], 1.0)

            mtw = wtp.tile([P, CT, C], F32R, tag="mtw")
            wvt = wtp.tile([P, CT, C], F32R, tag="wvt")
            pf = wtp.tile([P, CT, P], F32R, tag="pf")
            gsel = wtp.tile([P, N], F32R, tag="gsel")
            xf0 = xfp.tile([P, CT, N], F32R, tag="xf")
            for ct in range(CT):
                nc.sync.dma_start(mtw[:, ct], mt_ext[ct * P:(ct + 1) * P, :])
                nc.sync.dma_start(xf0[:, ct], x_ext[0, ct * P:(ct + 1) * P, :])
            for ct in range(CT):
                nc.sync.dma_start(wvt[:, ct], wvt_ext[ct * P:(ct + 1) * P, :])
                nc.sync.dma_start(pf[:, ct], pf_ext[ct * P:(ct + 1) * P, :])
            nc.sync.dma_start(gsel[:], gs_ext[:, :])

            for b in range(bloc):
                if b == 0:
                    xf = xf0
                else:
                    xf = xfp.tile([P, CT, N], F32R, tag="xf")
                    for ct in range(CT):
                        nc.sync.dma_start(xf[:, ct],
                                          x_ext[b, ct * P:(ct + 1) * P, :])

                # g = (Wq^T Wk) x  [c-part, ct, m] f32r
                g = qkp.tile([P, CT, N], F32R, tag="g")
                for oi in range(CT):
                    ps = pbig.tile([P, N], F32, tag="pbig")
                    for kt in range(CT):
                        for nb in range(2):
                            nc.tensor.matmul(
                                ps[:, nb * 512:(nb + 1) * 512],
                                mtw[:, kt, oi * P:(oi + 1) * P],
                                xf[:, kt, nb * 512:(nb + 1) * 512],
                                start=(kt == 0), stop=(kt == CT - 1),
                            )
                    nc.vector.tensor_copy(g[:, oi], ps[:])

                # t = PF^T x
                t = qkp.tile([P, N], F32R, tag="t")
                pst_t = pbig.tile([P, N], F32, tag="pbig")
                for kt in range(CT):
                    for nb in range(2):
                        nc.tensor.matmul(
                            pst_t[:, nb * 512:(nb + 1) * 512],
                            pf[:, kt],
                            xf[:, kt, nb * 512:(nb + 1) * 512],
                            start=(kt == 0), stop=(kt == CT - 1),
                        )
                nc.vector.tensor_copy(t[:], pst_t[:])

                # t-block^T via plain f32r identity matmul
                pstT = pbig.tile([P, N], F32, tag="pbig")
                for mt in range(NT):
                    nc.tensor.matmul(
                        pstT[:, mt * P:(mt + 1) * P],
                        t[:, mt * P:(mt + 1) * P],
                        ident_f32r,
                        start=True, stop=True,
                    )
                tT = qkp.tile([P, NT, P], F32, tag="tT")
                nc.vector.tensor_copy(
                    tT[:],
                    pstT[:].rearrange("p (j c) -> p j c", j=NT),
                )

                # v^T [m-part, mt, c] bf16
                vt = vtp.tile([P, NT, C], BF16, tag="vt")
                for mt in range(NT):
                    psv = pvp.tile([P, 512], F32, tag="pv")
                    for kt in range(CT):
                        nc.tensor.matmul(
                            psv[:],
                            xf[:, kt, mt * P:(mt + 1) * P],
                            wvt[:, kt],
                            start=(kt == 0), stop=(kt == CT - 1),
                        )
                    nc.vector.tensor_copy(vt[:, mt], psv[:])

                # S^T tiles + exp -> unnormalized E^T in out-matmul layout
                et = etp.tile([P, NT, N], BF16, tag="et")
                psR = psrp.tile([P, N], F32, tag="psr")
                for mt in range(NT):
                    psS = pbig.tile([P, N], F32, tag="pbig")
                    for mb in range(2):
                        for kt in range(CT):
                            nc.tensor.matmul(
                                psS[:, mb * 512:(mb + 1) * 512],
                                g[:, kt, mt * P:(mt + 1) * P],
                                xf[:, kt, mb * 512:(mb + 1) * 512],
                                start=(kt == 0), stop=(kt == CT - 1),
                            )
                    sf = sfp.tile([P, N], F32, tag="sf")
                    h_b = (tT[:, mt, 0:H]
                           .rearrange("p (j o) -> p j o", o=1)
                           .broadcast_to([P, H, W_]))
                    w_b = (tT[:, mt, H:2 * H]
                           .rearrange("p (o w) -> p o w", o=1)
                           .broadcast_to([P, H, W_]))
                    nc.vector.tensor_tensor(
                        sf[:].rearrange("p (j w) -> p j w", j=H),
                        psS[:].rearrange("p (j w) -> p j w", j=H),
                        h_b, mybir.AluOpType.add)
                    nc.gpsimd.tensor_tensor(
                        sf[:].rearrange("p (j w) -> p j w", j=H),
                        sf[:].rearrange("p (j w) -> p j w", j=H),
                        w_b, mybir.AluOpType.add)
                    nc.scalar.activation(et[:, mt], sf[:],
                                         mybir.ActivationFunctionType.Exp,
                                         bias=nbias[:], scale=1.0)
                    for nb in range(2):
                        nc.tensor.matmul(
                            psR[:, nb * 512:(nb + 1) * 512],
                            ones_bf[:],
                            et[:, mt, nb * 512:(nb + 1) * 512],
                            start=(mt == 0), stop=(mt == NT - 1),
                        )

                rrec = rpp.tile([P, N], F32, tag="rrec")
                nc.vector.reciprocal(rrec[:], psR[:])

                for ct in range(CT):
                    psO = pbig.tile([P, N], F32, tag="pbig")
                    for nb in range(2):
                        for mt in range(NT):
                            nc.tensor.matmul(
                                psO[:, nb * 512:(nb + 1) * 512],
                                vt[:, mt, ct * P:(ct + 1) * P],
                                et[:, mt, nb * 512:(nb + 1) * 512],
                                start=(mt == 0), stop=(mt == NT - 1),
                            )
                    ob = osbp.tile([P, N], F32, tag="osb")
                    nc.vector.tensor_tensor(ob[:], psO[:], rrec[:],
                                            mybir.AluOpType.mult)
                    nc.sync.dma_start(o_ext[b, ct * P:(ct + 1) * P, :], ob[:])

    nc.compile()
    return nc


_NC_CACHE = None


def _get_nc():
    global _NC_CACHE
    if _NC_CACHE is None:
        _NC_CACHE = build_nc()
    return _NC_CACHE


def _prep_inputs(x, W, rel_h, rel_w):
    x = np.ascontiguousarray(np.asarray(x, dtype=np.float32))
    W = np.asarray(W, dtype=np.float32).astype(np.float64)
    rel_h = np.asarray(rel_h, dtype=np.float32).reshape(C, H, 1)
    rel_w = np.asarray(rel_w, dtype=np.float32).reshape(C, 1, W_)
    Wq, Wk, Wv = W[0:C], W[C:2 * C], W[2 * C:3 * C]
    mt_h = _round_f32r((Wq.T @ Wk).T)       # lhsT layout [c', co]
    wvt_h = _round_f32r(Wv.T)               # [c, co]
    pf = np.zeros((C, P), np.float64)
    pf[:, 0:H] = Wq.T @ np.asarray(rel_h, np.float64).reshape(C, H)
    pf[:, H:2 * H] = Wq.T @ np.asarray(rel_w, np.float64).reshape(C, W_)
    pf_h = _round_f32r(pf)
    gs = np.zeros((P, N), np.float32)
    n_idx = np.arange(N)
    gs[n_idx // W_, n_idx] = 1.0
    gs[H + n_idx % W_, n_idx] = 1.0
    gs_h = _round_f32r(gs)
    xs = _round_f32r(x).reshape(NCORES, BLOC, C, N)
    return xs, mt_h, wvt_h, pf_h, gs_h


def kernel(x, W, rel_h, rel_w):
    nc = _get_nc()
    xs, mt_h, wvt_h, pf_h, gs_h = _prep_inputs(x, W, rel_h, rel_w)
    in_maps = [
        {"x": np.ascontiguousarray(xs[i]), "MT": mt_h, "WVT": wvt_h,
         "PF": pf_h, "GS": gs_h}
        for i in range(NCORES)
    ]
    res = run_bass_kernel_spmd(nc, in_maps, core_ids=list(range(NCORES)))
    out = np.concatenate([res.results[i]["out"] for i in range(NCORES)], axis=0)
    return out.reshape(B, C, H, W_)
